# revision 1
# baseline (speedup 1.0000x reference)
"""Bass/Trainium2 kernel for nn_Attention_47622597378289.

Two chained attention blocks (encoder, decoder) over [B=8, C=512, H=W=48].
Data-parallel over batch: core i handles batch item i (B == n_cores == 8).

Per-core computation (N = H*W = 2304, C8 = 64), all in [channel, pixel]
layouts chosen so every matmul contracts over the partition dim:

  Q  [64, N]   = WqT.T @ qsrc           (+ bq, via ACT bias)
  Kp [64, N]   = WkT.T @ kvsrc + pos    (pos includes bk, host-folded)
  VT [N, 512]  = kvsrc.T @ WvT.T.T      (bf16 in SBUF)
  ET [m, n]    = Kp.T @ Q               (PE, fp32r)
  A  = exp(ET)                          (ScalarE, bf16, no max subtract)
  S  [1, n]    = ones.T @ A             (PE, accumulated over m-chunks)
  OutT [n,512] = A.T @ VT               (PE, bf16, PSUM fp32 accum)
  res          = (gamma/S) * OutT + residual
"""

import numpy as np

import concourse.bass as bass
import concourse.bacc as bacc
import concourse.mybir as mybir
from concourse.bass_utils import run_bass_kernel_spmd
from concourse.masks import make_identity
from concourse.tile import TileContext

F32 = mybir.dt.float32
F32R = mybir.dt.float32r
BF16 = mybir.dt.bfloat16
AF = mybir.ActivationFunctionType
OP = mybir.AluOpType

B, C, H, W = 8, 512, 48, 48
C8 = C // 8          # 64
N = H * W            # 2304
P = 128
KC = C // P          # 4 c-chunks
NM = N // P          # 18 m/n chunks
# n handled in groups; each group is softmax-normalized + output independently.
# The small group goes first: its shorter m-loop ramps the E/exp/Out pipeline
# with less serial latency at each block start.
NGROUPS = [(2048, 256), (0, 512), (512, 512), (1024, 512), (1536, 512)]


def f32(ap):
    """Bitcast an fp32r AP back to plain fp32 for DVE/ACT consumers."""
    return ap.bitcast(F32)


def _attn_block(nc, tc, pools, wt, xs, q_src, out_mode, gamma, misc):
    """Emit one attention block.

    Group order: encoder puts the short 256-wide group first (cheap pipeline
    ramp at block start); decoder puts it last (short exposed tail before the
    kernel drain).

    q_src: dict(kind="sbuf", tile=) for resident [128, KC*N] source, or
           dict(kind="dram", t=) to stream [512, N] from DRAM.
    xs:    resident kv-source tile [128, KC*N] (f32).
    out_mode: ("enc", x_enc_tile) -> transpose back + residual from misc["xs"]
              ("dec", (xtd_dram, out_dram)) -> add x.T residual, DMA out.
    """
    pp_proj, pp_e, pp_out, pp_tr = (
        pools["pp_proj"], pools["pp_e"], pools["pp_out"], pools["pp_tr"],
    )
    sm = pools["small"]
    ident = misc["ident"]
    ones = misc["ones"]
    groups = NGROUPS if out_mode[0] == "enc" else NGROUPS[1:] + NGROUPS[:1]

    # ---- projections ----
    q_sb = pools["qk"].tile([C8, N], BF16, tag="q")
    kp_sb = pools["qk"].tile([C8, N], BF16, tag="kp")
    vt_sb = pools["vt"].tile([P, NM * C], BF16, tag="vt")

    for n0, nw in groups:
        kpp = pp_proj.tile([C8, 512], F32, tag="proj")
        for k in range(KC):
            nc.tensor.matmul(
                kpp[:, :nw],
                wt["wkT"][:, k * C8 : (k + 1) * C8],
                xs[:, k * N + n0 : k * N + n0 + nw],
                start=(k == 0),
                stop=(k == KC - 1),
            )
        nc.vector.tensor_add(
            kp_sb[:, n0 : n0 + nw], kpp[:, :nw], wt["pos"][:, n0 : n0 + nw]
        )

    # wvT is loaded lazily here (not with the other weights) so the startup
    # DMAs that gate the K projection aren't queued behind 1MB of wvT.
    wvT = wt["load_wvT"]()
    for mi in range(NM):
        vp = pp_proj.tile([P, C], F32, tag="proj")
        for k in range(KC):
            nc.tensor.matmul(
                vp,
                xs[:, k * N + mi * P : k * N + (mi + 1) * P],
                wvT[:, k * C : (k + 1) * C],
                start=(k == 0),
                stop=(k == KC - 1),
            )
        nc.vector.tensor_copy(vt_sb[:, mi * C : (mi + 1) * C], vp)

    # Q last: when q_src streams from DRAM the matmuls are DMA-paced, so they
    # must not hold pp_proj slots ahead of K/VT work.
    dma_rr = [nc.sync, nc.scalar]
    for ni, (n0, nw) in enumerate(groups):
        qp = pp_proj.tile([C8, 512], F32, tag="proj")
        for k in range(KC):
            if q_src["kind"] == "sbuf":
                rhs = q_src["tile"][:, k * N + n0 : k * N + n0 + nw]
            else:
                rhs_t = pools["stream"].tile([P, 512], F32R, tag="qstream")
                dma_rr[(ni * KC + k) % 2].dma_start(
                    out=rhs_t[:, :nw],
                    in_=q_src["t"][k * P : (k + 1) * P, n0 : n0 + nw],
                )
                rhs = rhs_t[:, :nw]
            nc.tensor.matmul(
                qp[:, :nw],
                wt["wqT"][:, k * C8 : (k + 1) * C8],
                rhs,
                start=(k == 0),
                stop=(k == KC - 1),
            )
        nc.vector.tensor_scalar(
            q_sb[:, n0 : n0 + nw], qp[:, :nw], wt["bq"][:, 0:1], None, OP.add
        )

    # ---- attention per n-group ----
    for n0, gw in groups:
        nsub = gw // P
        exp_sb = pools["expe"].tile([P, NM * 512], BF16, tag="expe")
        s_ps = pp_tr.tile([1, 512], F32, tag="tr", name="s_ps")
        for mi in range(NM):
            ep = pp_e.tile([P, 512], F32, tag="e")
            nc.tensor.matmul(
                ep[:, :gw],
                kp_sb[:, mi * P : (mi + 1) * P],
                q_sb[:, n0 : n0 + gw],
                start=True,
                stop=True,
            )
            nc.scalar.activation(
                exp_sb[:, mi * 512 : mi * 512 + gw], ep[:, :gw], AF.Exp
            )
            nc.tensor.matmul(
                s_ps[:, :gw],
                ones[:, 0:1],
                exp_sb[:, mi * 512 : mi * 512 + gw],
                start=(mi == 0),
                stop=(mi == NM - 1),
            )
        # S -> SBUF row, transpose to per-partition cols, THEN reciprocal so
        # the iterative divide runs on 128 lanes x nsub elems, not 1 x gw.
        s_row = sm.tile([1, 512], F32, tag="srow")
        nc.vector.tensor_copy(s_row[:, :gw], s_ps[:, :gw])
        s_cols = sm.tile([P, nsub], F32, tag="scol")
        for j in range(nsub):
            ftp = pp_tr.tile([P, P], F32, tag="tr")
            nc.tensor.transpose(
                ftp[:, 0:1], s_row[0:1, j * P : (j + 1) * P], ident[0:1, 0:1]
            )
            nc.vector.tensor_copy(s_cols[:, j : j + 1], ftp[:, 0:1])
        f_cols = sm.tile([P, nsub], F32, tag="fcol")
        nc.vector.reciprocal(f_cols, s_cols)
        nc.vector.tensor_scalar_mul(f_cols, f_cols, float(gamma))

        for j in range(nsub):
            op = pp_out.tile([P, C], F32, tag="out")
            for mi in range(NM):
                nc.tensor.matmul(
                    op,
                    exp_sb[:, mi * 512 + j * P : mi * 512 + (j + 1) * P],
                    vt_sb[:, mi * C : (mi + 1) * C],
                    start=(mi == 0),
                    stop=(mi == NM - 1),
                )
            rows0 = n0 + j * P
            if out_mode[0] == "enc":
                x_enc = out_mode[1]
                o_sb = pools["osb"].tile([P, C], F32, tag="osb")
                nc.vector.tensor_scalar_mul(o_sb, op, f_cols[:, j : j + 1])
                for k in range(KC):
                    trp = pp_tr.tile([P, P], F32, tag="tr")
                    nc.tensor.transpose(
                        trp, o_sb[:, k * P : (k + 1) * P], ident
                    )
                    nc.vector.scalar_tensor_tensor(
                        out=x_enc[:, k * N + rows0 : k * N + rows0 + P],
                        in0=trp,
                        scalar=misc["gvb"][:, k : k + 1],
                        in1=f32(misc["xs"][:, k * N + rows0 : k * N + rows0 + P]),
                        op0=OP.add,
                        op1=OP.add,
                    )
            else:
                xtd_dram, out_dram = out_mode[1]
                xtd_t = pools["stream"].tile([P, C], F32, tag="xtd")
                nc.gpsimd.dma_start(
                    out=xtd_t, in_=xtd_dram[rows0 : rows0 + P, :]
                )
                res_t = pools["osb"].tile([P, C], F32, tag="osb")
                nc.vector.scalar_tensor_tensor(
                    out=res_t,
                    in0=op,
                    scalar=f_cols[:, j : j + 1],
                    in1=xtd_t,
                    op0=OP.mult,
                    op1=OP.add,
                )
                nc.sync.dma_start(out=out_dram[rows0 : rows0 + P, :], in_=res_t)


def build_bass(gamma_e, gamma_d):
    nc = bacc.Bacc("TRN2", target_bir_lowering=False, debug=False)

    x_d = nc.dram_tensor("x_cn", [C, N], F32R, kind="ExternalInput")
    tot_d = nc.dram_tensor("tot_cn", [C, N], F32R, kind="ExternalInput")
    xtd_d = nc.dram_tensor("xTd", [N, C], F32, kind="ExternalInput")
    wts_d = {}
    for p in ("e", "d"):
        wts_d[p] = {
            "wqT": nc.dram_tensor(f"wqT_{p}", [P, KC * C8], F32R, kind="ExternalInput"),
            "wkT": nc.dram_tensor(f"wkT_{p}", [P, KC * C8], F32R, kind="ExternalInput"),
            "wvT": nc.dram_tensor(f"wvT_{p}", [P, KC * C], F32R, kind="ExternalInput"),
            "pos": nc.dram_tensor(f"pos_{p}", [C8, N], F32, kind="ExternalInput"),
            "bq": nc.dram_tensor(f"bq_{p}", [C8, 1], F32, kind="ExternalInput"),
        }
    gvb_d = nc.dram_tensor("gvb_e", [P, KC], F32, kind="ExternalInput")
    out_d = nc.dram_tensor("outT", [N, C], F32, kind="ExternalOutput")

    with TileContext(nc) as tc:
        import contextlib

        with contextlib.ExitStack() as ctx:
            pools = {
                "persist": ctx.enter_context(tc.tile_pool(name="persist", bufs=1)),
                "qk": ctx.enter_context(tc.tile_pool(name="qk", bufs=2)),
                "vt": ctx.enter_context(tc.tile_pool(name="vt", bufs=2)),
                "expe": ctx.enter_context(tc.tile_pool(name="expe", bufs=2)),
                "stream": ctx.enter_context(tc.tile_pool(name="stream", bufs=4)),
                "osb": ctx.enter_context(tc.tile_pool(name="osb", bufs=3)),
                "small": ctx.enter_context(tc.tile_pool(name="small", bufs=2)),
                "wpool": ctx.enter_context(tc.tile_pool(name="wpool", bufs=1)),
                "pp_proj": ctx.enter_context(
                    tc.tile_pool(name="pp_proj", bufs=2, space="PSUM")
                ),
                "pp_e": ctx.enter_context(
                    tc.tile_pool(name="pp_e", bufs=3, space="PSUM")
                ),
                "pp_out": ctx.enter_context(
                    tc.tile_pool(name="pp_out", bufs=2, space="PSUM")
                ),
                "pp_tr": ctx.enter_context(
                    tc.tile_pool(name="pp_tr", bufs=1, space="PSUM")
                ),
            }

            persist = pools["persist"]
            wpool = pools["wpool"]

            ident = wpool.tile([P, P], F32, tag="ident")
            make_identity(nc, ident)
            ones = wpool.tile([P, 1], BF16, tag="ones")
            nc.vector.memset(ones, 1.0)

            xs = persist.tile([P, KC * N], F32R, tag="xs")
            x_enc = persist.tile([P, KC * N], F32R, tag="x_enc")
            gvb = wpool.tile([P, KC], F32, tag="gvb")
            nc.gpsimd.dma_start(out=gvb, in_=gvb_d[:, :])

            def load_weights(p):
                # enc/dec share slots (same tags); dec's DMAs are emitted in
                # program order after the enc block so they only wait on enc's
                # last weight reads. wvT is deferred (load_wvT) so the 1MB
                # transfer doesn't delay the startup-critical Q/K weights.
                w = {
                    "wqT": wpool.tile([P, KC * C8], F32R, tag="wqT", name=f"wqT_{p}_sb"),
                    "wkT": wpool.tile([P, KC * C8], F32R, tag="wkT", name=f"wkT_{p}_sb"),
                    "pos": wpool.tile([C8, N], F32, tag="pos", name=f"pos_{p}_sb"),
                    "bq": wpool.tile([C8, 1], F32, tag="bq", name=f"bq_{p}_sb"),
                }
                nc.sync.dma_start(out=w["wkT"], in_=wts_d[p]["wkT"][:, :])
                nc.gpsimd.dma_start(out=w["bq"], in_=wts_d[p]["bq"][:, :])
                nc.gpsimd.dma_start(out=w["wqT"], in_=wts_d[p]["wqT"][:, :])
                nc.gpsimd.dma_start(out=w["pos"], in_=wts_d[p]["pos"][:, :])

                def load_wvT():
                    wv = wpool.tile(
                        [P, KC * C], F32R, tag="wvT", name=f"wvT_{p}_sb"
                    )
                    nc.sync.dma_start(
                        out=wv[:, 0 : 2 * C], in_=wts_d[p]["wvT"][:, 0 : 2 * C]
                    )
                    nc.scalar.dma_start(
                        out=wv[:, 2 * C : KC * C],
                        in_=wts_d[p]["wvT"][:, 2 * C : KC * C],
                    )
                    return wv

                w["load_wvT"] = load_wvT
                return w

            misc = {"ident": ident, "ones": ones, "gvb": gvb, "xs": xs}

            wt_e = load_weights("e")
            # xs after wkT on the sync ring (first K matmul needs both).
            # n-quartered so K/VT matmuls on early columns can start after
            # ~1.2MB instead of the full 4.7MB; c-chunks split across the
            # two HWDGE rings (sync + scalar).
            NQ = N // 4
            # quarter order matches K-proj's NGROUPS consumption order
            # (the 256-wide ramp group at n0=2048 comes first)
            for q in (3, 0, 1, 2):
                for k in range(KC):
                    eng = nc.sync if k % 2 == 0 else nc.scalar
                    eng.dma_start(
                        out=xs[:, k * N + q * NQ : k * N + (q + 1) * NQ],
                        in_=x_d[k * P : (k + 1) * P, q * NQ : (q + 1) * NQ],
                    )
            _attn_block(
                nc, tc, pools, wt_e, xs,
                {"kind": "dram", "t": tot_d},
                ("enc", x_enc), gamma_e, misc,
            )
            wt_d = load_weights("d")
            _attn_block(
                nc, tc, pools, wt_d, x_enc,
                {"kind": "sbuf", "tile": xs},
                ("dec", (xtd_d, out_d)), gamma_d, misc,
            )

    nc.compile()
    return nc


def kernel(**inputs):
    x = np.asarray(inputs["x"], np.float32)
    total = np.asarray(inputs["total"], np.float32)

    def prep(pfx):
        Wq = np.asarray(inputs[f"{pfx}_Wq"], np.float32)
        bq = np.asarray(inputs[f"{pfx}_bq"], np.float32)
        Wk = np.asarray(inputs[f"{pfx}_Wk"], np.float32)
        bk = np.asarray(inputs[f"{pfx}_bk"], np.float32)
        Wv = np.asarray(inputs[f"{pfx}_Wv"], np.float32)
        bv = np.asarray(inputs[f"{pfx}_bv"], np.float32)
        ht = np.asarray(inputs[f"{pfx}_ht"], np.float32)
        wtt = np.asarray(inputs[f"{pfx}_wt"], np.float32)
        gamma = float(np.asarray(inputs[f"{pfx}_gamma"], np.float32).reshape(-1)[0])
        pos = (ht + wtt).reshape(C8, N) + bk[:, None]
        def pack(wT):
            # [C, X] -> [128, KC*X]: c-chunk k at columns [k*X, (k+1)*X)
            X = wT.shape[1]
            out = np.empty((P, KC * X), np.float32)
            for k in range(KC):
                out[:, k * X : (k + 1) * X] = wT[k * P : (k + 1) * P]
            return out

        return {
            "wqT": pack(np.ascontiguousarray(Wq.T)),
            "wkT": pack(np.ascontiguousarray(Wk.T)),
            "wvT": pack(np.ascontiguousarray(Wv.T)),
            "pos": np.ascontiguousarray(pos),
            "bq": np.ascontiguousarray(bq.reshape(C8, 1)),
            "bv": bv,
            "gamma": gamma,
        }

    pe, pd = prep("enc"), prep("dec")
    gvb_e = (pe["gamma"] * np.asarray(inputs["enc_bv"], np.float32)).reshape(
        KC, P
    ).T  # [128, 4], col k = gamma_e*bv_e[k*128:(k+1)*128]
    gvb_e = np.ascontiguousarray(gvb_e)

    nc = build_bass(pe["gamma"], pd["gamma"])

    in_maps = []
    for b in range(B):
        x_cn = np.ascontiguousarray(x[b].reshape(C, N))
        tot_cn = np.ascontiguousarray(total[b].reshape(C, N))
        xtd = np.ascontiguousarray(
            x_cn.T + pd["gamma"] * np.asarray(inputs["dec_bv"], np.float32)[None, :]
        )
        m = {
            "x_cn": x_cn,
            "tot_cn": tot_cn,
            "xTd": xtd,
            "gvb_e": gvb_e,
        }
        for p, w in (("e", pe), ("d", pd)):
            m[f"wqT_{p}"] = w["wqT"]
            m[f"wkT_{p}"] = w["wkT"]
            m[f"wvT_{p}"] = w["wvT"]
            m[f"pos_{p}"] = w["pos"]
            m[f"bq_{p}"] = w["bq"]
        in_maps.append(m)

    res = run_bass_kernel_spmd(nc, in_maps, core_ids=list(range(B)))
    out = np.stack(
        [res.results[b]["outT"].T.reshape(C, H, W) for b in range(B)], axis=0
    )
    return out.astype(np.float32)


if __name__ == "__main__":
    import reference

    ins = {k: np.asarray(v) for k, v in reference.setup_inputs().items()}
    got = kernel(**ins)
    exp = np.asarray(reference.reference(**ins))
    err = np.abs(got - exp).max() / (np.abs(exp).max() + 1e-30)
    print("abs-rel err:", err)



# revision 3
# speedup vs baseline: 1.0907x; 1.0907x over previous
"""Bass/Trainium2 kernel for nn_Attention_47622597378289.

Two chained attention blocks (encoder, decoder) over [B=8, C=512, H=W=48].
Data-parallel over batch: core i handles batch item i (B == n_cores == 8).

v2 design (vs v0 baseline):
  - All matmul operands bf16 (host-cast): no fp32 HIGH/LOW_HIGH PE modes,
    FWL-fast weight loads throughout.
  - Out matmul computed TRANSPOSED: out[c,n] = sum_m vt[m,c].T @ exp[m,n].
    Kills all 72 PE transposes, produces x_enc directly in [c,n] layout for
    the decoder, and lets Out consume exp chunks as they are produced
    (per-mi pipeline: ScalarE exp fully hidden under PE work).
  - E matmuls row-packed 2x via tile_position (K=C8=64): kp/q stored with
    duplicated partition halves so two m-chunks stream concurrently.
  - S computed with an all-ones [128,128] lhsT -> arrives pre-broadcast
    [128,gw]; 1/S via DVE reciprocal_approx_fast; gamma folded into vt.
  - Per-group normalize: out*f (+gamma*bv per-channel) + residual via two
    DVE ops; decoder residual/output streamed [c,n] (no xtd transpose
    stream, no output transpose).
"""

import numpy as np

import concourse.bass as bass
import concourse.bacc as bacc
import concourse.mybir as mybir
from concourse.bass_utils import run_bass_kernel_spmd
from concourse.tile import TileContext

F32 = mybir.dt.float32
BF16 = mybir.dt.bfloat16
AF = mybir.ActivationFunctionType
OP = mybir.AluOpType

B, C, H, W = 8, 512, 48, 48
C8 = C // 8          # 64
N = H * W            # 2304
P = 128
KC = C // P          # 4 c-chunks
NM = N // P          # 18 m-chunks
GROUPS = [(0, 512), (512, 512), (1024, 512), (1536, 512), (2048, 256)]
NQ = N // 4          # 576-col load quarters


def build_bass(gamma_e, gamma_d):
    nc = bacc.Bacc("TRN2", target_bir_lowering=False, debug=False)

    xbf_d = nc.dram_tensor("x_bf", [P, KC * N], BF16, kind="ExternalInput")
    tbf_d = nc.dram_tensor("tot_bf", [P, KC * N], BF16, kind="ExternalInput")
    xf_d = nc.dram_tensor("x_cn", [C, N], F32, kind="ExternalInput")
    wts_d = {}
    for p in ("e", "d"):
        wts_d[p] = {
            "wk": nc.dram_tensor(f"wk_{p}", [P, KC * P], BF16, kind="ExternalInput"),
            "wq": nc.dram_tensor(f"wq_{p}", [P, KC * P], BF16, kind="ExternalInput"),
            "wv": nc.dram_tensor(f"wv_{p}", [P, KC * C], BF16, kind="ExternalInput"),
            "pos": nc.dram_tensor(f"pos_{p}", [P, N], F32, kind="ExternalInput"),
            "bq": nc.dram_tensor(f"bq_{p}", [P, 1], F32, kind="ExternalInput"),
            "gvb": nc.dram_tensor(f"gvb_{p}", [P, KC], F32, kind="ExternalInput"),
        }
    out_d = nc.dram_tensor("out_cn", [C, N], F32, kind="ExternalOutput")

    with TileContext(nc) as tc:
        import contextlib

        with contextlib.ExitStack() as ctx:
            pl = {
                "persist": ctx.enter_context(tc.tile_pool(name="persist", bufs=1)),
                "wpool": ctx.enter_context(tc.tile_pool(name="wpool", bufs=1)),
                "kq": ctx.enter_context(tc.tile_pool(name="kq", bufs=2)),
                "vt": ctx.enter_context(tc.tile_pool(name="vt", bufs=2)),
                "expe": ctx.enter_context(tc.tile_pool(name="expe", bufs=2)),
                "fbc": ctx.enter_context(tc.tile_pool(name="fbc", bufs=2)),
                "osb": ctx.enter_context(tc.tile_pool(name="osb", bufs=6)),
                "stream": ctx.enter_context(tc.tile_pool(name="stream", bufs=4)),
                "pp_e": ctx.enter_context(
                    tc.tile_pool(name="pp_e", bufs=3, space="PSUM")
                ),
                "pp_out": ctx.enter_context(
                    tc.tile_pool(name="pp_out", bufs=4, space="PSUM")
                ),
                "pp_s": ctx.enter_context(
                    tc.tile_pool(name="pp_s", bufs=1, space="PSUM")
                ),
            }
            wpool = pl["wpool"]
            persist = pl["persist"]

            ones = wpool.tile([P, P], BF16, tag="ones")
            nc.vector.memset(ones, 1.0)

            xs_bf = persist.tile([P, KC * N], BF16, tag="xs_bf")
            tot_bf = persist.tile([P, KC * N], BF16, tag="tot_bf")
            xenc_bf = persist.tile([P, KC * N], BF16, tag="xenc_bf")

            def load_weights(p):
                w = {
                    "wk": wpool.tile([P, KC * P], BF16, tag="wk", name=f"wk_{p}_sb"),
                    "wq": wpool.tile([P, KC * P], BF16, tag="wq", name=f"wq_{p}_sb"),
                    "wv": wpool.tile([P, KC * C], BF16, tag="wv", name=f"wv_{p}_sb"),
                    "pos": wpool.tile([P, N], F32, tag="pos", name=f"pos_{p}_sb"),
                    "bq": wpool.tile([P, 1], F32, tag="bq", name=f"bq_{p}_sb"),
                    "gvb": wpool.tile([P, KC], F32, tag="gvb", name=f"gvb_{p}_sb"),
                }
                # wv first (V-proj runs first); small tensors on gpsimd ring.
                nc.scalar.dma_start(out=w["wv"], in_=wts_d[p]["wv"][:, :])
                nc.gpsimd.dma_start(out=w["wk"], in_=wts_d[p]["wk"][:, :])
                nc.gpsimd.dma_start(out=w["wq"], in_=wts_d[p]["wq"][:, :])
                nc.gpsimd.dma_start(out=w["bq"], in_=wts_d[p]["bq"][:, :])
                nc.gpsimd.dma_start(out=w["gvb"], in_=wts_d[p]["gvb"][:, :])
                nc.scalar.dma_start(out=w["pos"], in_=wts_d[p]["pos"][:, :])
                return w

            wt_e = load_weights("e")

            # x (bf16, packed [128, KC*N]) in quarters, k-chunks split over
            # the sync+scalar rings; V-proj consumption interleaves below.
            for q in range(4):
                for k in range(KC):
                    eng = nc.sync if k % 2 == 0 else nc.scalar
                    eng.dma_start(
                        out=xs_bf[:, k * N + q * NQ : k * N + (q + 1) * NQ],
                        in_=xbf_d[:, k * N + q * NQ : k * N + (q + 1) * NQ],
                    )
            for q in range(4):
                for k in range(KC):
                    eng = nc.sync if k % 2 == 0 else nc.scalar
                    eng.dma_start(
                        out=tot_bf[:, k * N + q * NQ : k * N + (q + 1) * NQ],
                        in_=tbf_d[:, k * N + q * NQ : k * N + (q + 1) * NQ],
                    )

            def proj_v(vt, wv, src, gamma):
                # vt chunk (mi,kc) at cols (mi*KC+kc)*P; mi block contiguous.
                for mi in range(NM):
                    vp = pl["pp_e"].tile([P, C], F32, tag="e", name=f"vp{mi}")
                    for k in range(KC):
                        nc.tensor.matmul(
                            vp,
                            src[:, k * N + mi * P : k * N + (mi + 1) * P],
                            wv[:, k * C : (k + 1) * C],
                            start=(k == 0),
                            stop=(k == KC - 1),
                        )
                    nc.vector.tensor_scalar_mul(
                        vt[:, mi * C : (mi + 1) * C], vp, float(gamma)
                    )

            def proj_k(kp, wk, pos, src, tiles=None):
                for t, (n0, nw) in enumerate(GROUPS if tiles is None else tiles):
                    pp = pl["pp_e"].tile([P, 512], F32, tag="e", name="kpp")
                    for k in range(KC):
                        nc.tensor.matmul(
                            pp[:, :nw],
                            wk[:, k * P : (k + 1) * P],
                            src[:, k * N + n0 : k * N + n0 + nw],
                            start=(k == 0),
                            stop=(k == KC - 1),
                        )
                    nc.vector.tensor_add(
                        kp[:, n0 : n0 + nw], pp[:, :nw], pos[:, n0 : n0 + nw]
                    )

            def proj_q(qt, wq, bq, src):
                for t, (n0, nw) in enumerate(GROUPS):
                    pp = pl["pp_e"].tile([P, 512], F32, tag="e", name="qpp")
                    for k in range(KC):
                        nc.tensor.matmul(
                            pp[:, :nw],
                            wq[:, k * P : (k + 1) * P],
                            src[:, k * N + n0 : k * N + n0 + nw],
                            start=(k == 0),
                            stop=(k == KC - 1),
                        )
                    nc.scalar.activation(
                        qt[:, n0 : n0 + nw], pp[:, :nw], AF.Identity, bias=bq
                    )

            def attn(kp, q, vt, wt, mode, post_group=None):
                """mode: ("enc",) writes xenc_bf; ("dec",) DMAs out_cn."""
                enc = mode == "enc"
                for gi, (n0, gw) in enumerate(GROUPS):
                    exp_sb = pl["expe"].tile(
                        [P, NM * 512], BF16, tag="expe", name=f"exp_{mode}{gi}"
                    )
                    s_ps = pl["pp_s"].tile([P, 512], F32, tag="s", name="s_ps")
                    ops = [
                        pl["pp_out"].tile([P, 512], F32, tag="out", name=f"o{kc}")
                        for kc in range(KC)
                    ]
                    res_t = []
                    if not enc:
                        for kc in range(KC):
                            rt = pl["stream"].tile(
                                [P, 512], F32, tag="res", name=f"res{kc}"
                            )
                            nc.gpsimd.dma_start(
                                out=rt[:, :gw],
                                in_=xf_d[kc * P : (kc + 1) * P, n0 : n0 + gw],
                            )
                            res_t.append(rt)
                    for pr in range(0, NM, 2):
                        ea = pl["pp_e"].tile([P, 512], F32, tag="e", name="ea")
                        eb = pl["pp_e"].tile([P, 512], F32, tag="e", name="eb")
                        nc.tensor.matmul(
                            ea[:, :gw],
                            kp[0:C8, pr * P : (pr + 1) * P],
                            q[0:C8, n0 : n0 + gw],
                            start=True,
                            stop=True,
                        )
                        nc.tensor.matmul(
                            eb[:, :gw],
                            kp[C8:P, (pr + 1) * P : (pr + 2) * P],
                            q[C8:P, n0 : n0 + gw],
                            start=True,
                            stop=True,
                        )
                        sl0 = pr * 512
                        sl1 = (pr + 1) * 512
                        nc.scalar.activation(
                            exp_sb[:, sl0 : sl0 + gw], ea[:, :gw], AF.Exp
                        )
                        nc.scalar.activation(
                            exp_sb[:, sl1 : sl1 + gw], eb[:, :gw], AF.Exp
                        )
                        nc.tensor.matmul(
                            s_ps[:, :gw],
                            ones,
                            exp_sb[:, sl0 : sl0 + gw],
                            start=(pr == 0),
                            stop=False,
                        )
                        nc.tensor.matmul(
                            s_ps[:, :gw],
                            ones,
                            exp_sb[:, sl1 : sl1 + gw],
                            start=False,
                            stop=(pr == NM - 2),
                        )
                        for kc in range(KC):
                            nc.tensor.matmul(
                                ops[kc][:, :gw],
                                vt[:, (pr * KC + kc) * P : (pr * KC + kc + 1) * P],
                                exp_sb[:, sl0 : sl0 + gw],
                                start=(pr == 0),
                                stop=False,
                            )
                            nc.tensor.matmul(
                                ops[kc][:, :gw],
                                vt[:, ((pr + 1) * KC + kc) * P : ((pr + 1) * KC + kc + 1) * P],
                                exp_sb[:, sl1 : sl1 + gw],
                                start=False,
                                stop=(pr == NM - 2),
                            )
                    fb = pl["fbc"].tile([P, 512], F32, tag="fbc", name="fbc")
                    nc.vector.reciprocal_approx_fast(fb[:, :gw], s_ps[:, :gw])
                    for kc in range(KC):
                        t1 = pl["osb"].tile([P, 512], F32, tag="osb", name=f"t1_{kc}")
                        nc.vector.tensor_mul(
                            t1[:, :gw], ops[kc][:, :gw], fb[:, :gw]
                        )
                        if enc:
                            nc.vector.scalar_tensor_tensor(
                                out=xenc_bf[:, kc * N + n0 : kc * N + n0 + gw],
                                in0=t1[:, :gw],
                                scalar=wt["gvb"][:, kc : kc + 1],
                                in1=xs_bf[:, kc * N + n0 : kc * N + n0 + gw],
                                op0=OP.add,
                                op1=OP.add,
                            )
                        else:
                            ro = pl["osb"].tile(
                                [P, 512], F32, tag="osb", name=f"ro_{kc}"
                            )
                            nc.vector.scalar_tensor_tensor(
                                out=ro[:, :gw],
                                in0=t1[:, :gw],
                                scalar=wt["gvb"][:, kc : kc + 1],
                                in1=res_t[kc][:, :gw],
                                op0=OP.add,
                                op1=OP.add,
                            )
                            nc.sync.dma_start(
                                out=out_d[kc * P : (kc + 1) * P, n0 : n0 + gw],
                                in_=ro[:, :gw],
                            )
                    if post_group is not None:
                        post_group(gi, n0, gw)

            # ---- encoder ----
            vt_e = pl["vt"].tile([P, NM * C], BF16, tag="vt", name="vt_e")
            kp_e = pl["kq"].tile([P, N], BF16, tag="kp", name="kp_e")
            q_e = pl["kq"].tile([P, N], BF16, tag="q", name="q_e")
            proj_v(vt_e, wt_e["wv"], xs_bf, gamma_e)
            proj_k(kp_e, wt_e["wk"], wt_e["pos"], xs_bf)
            proj_q(q_e, wt_e["wq"], wt_e["bq"], tot_bf)

            wt_d = load_weights("d")
            vt_d = pl["vt"].tile([P, NM * C], BF16, tag="vt", name="vt_d")
            kp_d = pl["kq"].tile([P, N], BF16, tag="kp", name="kp_d")
            q_d = pl["kq"].tile([P, N], BF16, tag="q", name="q_d")
            proj_q(q_d, wt_d["wq"], wt_d["bq"], xs_bf)

            def enc_post(gi, n0, gw):
                # dec projections that depend on this x_enc slice
                proj_k(kp_d, wt_d["wk"], wt_d["pos"], xenc_bf, tiles=[(n0, gw)])
                for mi in range(n0 // P, (n0 + gw) // P):
                    vp = pl["pp_e"].tile([P, C], F32, tag="e", name=f"dvp{mi}")
                    for k in range(KC):
                        nc.tensor.matmul(
                            vp,
                            xenc_bf[:, k * N + mi * P : k * N + (mi + 1) * P],
                            wt_d["wv"][:, k * C : (k + 1) * C],
                            start=(k == 0),
                            stop=(k == KC - 1),
                        )
                    nc.vector.tensor_scalar_mul(
                        vt_d[:, mi * C : (mi + 1) * C], vp, float(gamma_d)
                    )

            attn(kp_e, q_e, vt_e, wt_e, "enc", post_group=enc_post)

            # ---- decoder ----
            attn(kp_d, q_d, vt_d, wt_d, "dec")

    nc.compile()
    return nc


def kernel(**inputs):
    x = np.asarray(inputs["x"], np.float32)
    total = np.asarray(inputs["total"], np.float32)

    def pack_cn(a_cn):
        out = np.empty((P, KC * a_cn.shape[1]), a_cn.dtype)
        M = a_cn.shape[1]
        for k in range(KC):
            out[:, k * M : (k + 1) * M] = a_cn[k * P : (k + 1) * P]
        return out

    def prep(pfx):
        Wq = np.asarray(inputs[f"{pfx}_Wq"], np.float32)
        bq = np.asarray(inputs[f"{pfx}_bq"], np.float32)
        Wk = np.asarray(inputs[f"{pfx}_Wk"], np.float32)
        bk = np.asarray(inputs[f"{pfx}_bk"], np.float32)
        Wv = np.asarray(inputs[f"{pfx}_Wv"], np.float32)
        bv = np.asarray(inputs[f"{pfx}_bv"], np.float32)
        ht = np.asarray(inputs[f"{pfx}_ht"], np.float32)
        wtt = np.asarray(inputs[f"{pfx}_wt"], np.float32)
        gamma = float(np.asarray(inputs[f"{pfx}_gamma"], np.float32).reshape(-1)[0])
        pos = (ht + wtt).reshape(C8, N) + bk[:, None]

        def dup_pack(wT):
            # [C, C8] -> [128, KC*128]: chunk k cols = [wT_k | wT_k]
            out = np.empty((P, KC * P), np.float32)
            for k in range(KC):
                blk = wT[k * P : (k + 1) * P]            # [128, 64]
                out[:, k * P : k * P + C8] = blk
                out[:, k * P + C8 : (k + 1) * P] = blk
            return out.astype(np.float32)

        pos_dup = np.concatenate([pos, pos], axis=0)      # [128, N]
        bq_dup = np.concatenate([bq, bq]).reshape(P, 1)
        gvb = np.ascontiguousarray((gamma * bv).reshape(KC, P).T)  # [128, KC]
        return {
            "wk": to_bf(dup_pack(np.ascontiguousarray(Wk.T))),
            "wq": to_bf(dup_pack(np.ascontiguousarray(Wq.T))),
            "wv": to_bf(pack_cn(np.ascontiguousarray(Wv.T))),
            "pos": np.ascontiguousarray(pos_dup),
            "bq": np.ascontiguousarray(bq_dup),
            "gvb": gvb,
            "gamma": gamma,
        }

    import ml_dtypes

    def to_bf(a):
        return a.astype(ml_dtypes.bfloat16)

    pe, pd = prep("enc"), prep("dec")
    nc = build_bass(pe["gamma"], pd["gamma"])

    in_maps = []
    for b in range(B):
        x_cn = np.ascontiguousarray(x[b].reshape(C, N))
        tot_cn = np.ascontiguousarray(total[b].reshape(C, N))
        m = {
            "x_bf": to_bf(pack_cn(x_cn)),
            "tot_bf": to_bf(pack_cn(tot_cn)),
            "x_cn": x_cn,
        }
        for p, w in (("e", pe), ("d", pd)):
            for key in ("wk", "wq", "wv", "pos", "bq", "gvb"):
                m[f"{key}_{p}"] = w[key]
        in_maps.append(m)

    res = run_bass_kernel_spmd(nc, in_maps, core_ids=list(range(B)))
    out = np.stack(
        [res.results[b]["out_cn"].reshape(C, H, W) for b in range(B)], axis=0
    )
    return out.astype(np.float32)


if __name__ == "__main__":
    import reference

    ins = {k: np.asarray(v) for k, v in reference.setup_inputs().items()}
    got = kernel(**ins)
    exp = np.asarray(reference.reference(**ins))
    err = np.abs(got - exp).max() / (np.abs(exp).max() + 1e-30)
    print("abs-rel err:", err)


# revision 6
# speedup vs baseline: 1.1452x; 1.0500x over previous
"""Bass/Trainium2 kernel for nn_Attention_47622597378289.

Two chained attention blocks (encoder, decoder) over [B=8, C=512, H=W=48].
Data-parallel over batch: core i handles batch item i (B == n_cores == 8).

v3 design:
  - All matmul operands bf16 (host-cast): no fp32 HIGH/LOW_HIGH PE modes,
    FWL-fast weight loads throughout.
  - Out matmul computed TRANSPOSED: out[c,n] = sum_m vt[m,c].T @ exp[m,n].
    No PE transposes, x_enc lands directly in [c,n] layout for the decoder,
    and Out consumes exp chunks as they are produced (per-mi pipeline:
    ScalarE exp hides under PE work).
  - E matmuls row-packed 2x via tile_position (K=C8=64) with duplicated
    kp/q partition halves.
  - S via all-ones [128,128] lhsT -> arrives pre-broadcast [128,gw];
    1/S with DVE reciprocal_approx_fast; gamma folded into vt, gamma*bv
    folded as per-partition scalar in the residual STT.
  - Startup DMA round-robined over 3 HWDGE rings (~118GB/s each), wv
    first (V-proj is the first consumer), pos in bf16.
  - Decoder projections emitted one group LATE so the DVE/GpSimd residual
    ops for x_enc finish before PE needs them; residual STT on GpSimd.
"""

import numpy as np

import concourse.bass as bass
import concourse.bacc as bacc
import concourse.mybir as mybir
from concourse.bass_utils import run_bass_kernel_spmd
from concourse.tile import TileContext

F32 = mybir.dt.float32
BF16 = mybir.dt.bfloat16
AF = mybir.ActivationFunctionType
OP = mybir.AluOpType

B, C, H, W = 8, 512, 48, 48
C8 = C // 8          # 64
N = H * W            # 2304
P = 128
KC = C // P          # 4 c-chunks
NM = N // P          # 18 m-chunks
GROUPS = [(0, 512), (512, 512), (1024, 512), (1536, 512), (2048, 256)]
NQ = N // 4          # 576-col load quarters


def build_bass(gamma_e, gamma_d):
    nc = bacc.Bacc("TRN2", target_bir_lowering=False, debug=False)

    xbf_d = nc.dram_tensor("x_bf", [P, KC * N], BF16, kind="ExternalInput")
    tbf_d = nc.dram_tensor("tot_bf", [P, KC * N], BF16, kind="ExternalInput")
    xf_d = nc.dram_tensor("x_cn", [C, N], F32, kind="ExternalInput")
    wts_d = {}
    for p in ("e", "d"):
        wts_d[p] = {
            "wk": nc.dram_tensor(f"wk_{p}", [P, KC * P], BF16, kind="ExternalInput"),
            "wq": nc.dram_tensor(f"wq_{p}", [P, KC * P], BF16, kind="ExternalInput"),
            "wv": nc.dram_tensor(f"wv_{p}", [P, KC * C], BF16, kind="ExternalInput"),
            "pos": nc.dram_tensor(f"pos_{p}", [P, N], BF16, kind="ExternalInput"),
            "bq": nc.dram_tensor(f"bq_{p}", [P, 1], F32, kind="ExternalInput"),
            "gvb": nc.dram_tensor(f"gvb_{p}", [P, KC], F32, kind="ExternalInput"),
        }
    out_d = nc.dram_tensor("out_cn", [C, N], F32, kind="ExternalOutput")

    with TileContext(nc) as tc:
        import contextlib

        with contextlib.ExitStack() as ctx:
            pl = {
                "persist": ctx.enter_context(tc.tile_pool(name="persist", bufs=1)),
                "wpool": ctx.enter_context(tc.tile_pool(name="wpool", bufs=1)),
                "kq": ctx.enter_context(tc.tile_pool(name="kq", bufs=2)),
                "vt": ctx.enter_context(tc.tile_pool(name="vt", bufs=2)),
                "expe": ctx.enter_context(tc.tile_pool(name="expe", bufs=2)),
                "fbc": ctx.enter_context(tc.tile_pool(name="fbc", bufs=2)),
                "osb": ctx.enter_context(tc.tile_pool(name="osb", bufs=6)),
                "stream": ctx.enter_context(tc.tile_pool(name="stream", bufs=4)),
                "pp_e": ctx.enter_context(
                    tc.tile_pool(name="pp_e", bufs=3, space="PSUM")
                ),
                "pp_out": ctx.enter_context(
                    tc.tile_pool(name="pp_out", bufs=4, space="PSUM")
                ),
                "pp_s": ctx.enter_context(
                    tc.tile_pool(name="pp_s", bufs=1, space="PSUM")
                ),
            }
            wpool = pl["wpool"]
            persist = pl["persist"]

            ones = wpool.tile([P, P], BF16, tag="ones")
            nc.vector.memset(ones, 1.0)

            xs_bf = persist.tile([P, KC * N], BF16, tag="xs_bf")
            tot_bf = persist.tile([P, KC * N], BF16, tag="tot_bf")
            xenc_bf = persist.tile([P, KC * N], BF16, tag="xenc_bf")

            rings = [nc.sync, nc.scalar, nc.gpsimd]
            ring_i = [0]

            def dma_rr(out, in_):
                rings[ring_i[0] % 3].dma_start(out=out, in_=in_)
                ring_i[0] += 1

            def wtiles(p):
                return {
                    "wk": wpool.tile([P, KC * P], BF16, tag=f"wk{p}", name=f"wk_{p}"),
                    "wq": wpool.tile([P, KC * P], BF16, tag=f"wq{p}", name=f"wq_{p}"),
                    "wv": wpool.tile([P, KC * C], BF16, tag=f"wv{p}", name=f"wv_{p}"),
                    "pos": wpool.tile([P, N], BF16, tag=f"pos{p}", name=f"pos_{p}"),
                    "bq": wpool.tile([P, 1], F32, tag=f"bq{p}", name=f"bq_{p}"),
                    "gvb": wpool.tile([P, KC], F32, tag=f"gvb{p}", name=f"gvb_{p}"),
                }

            def load_wv(w, p):
                for c0, c1 in ((0, 768), (768, 1536), (1536, KC * C)):
                    dma_rr(w["wv"][:, c0:c1], wts_d[p]["wv"][:, c0:c1])

            def load_small(w, p):
                dma_rr(w["wk"], wts_d[p]["wk"][:, :])
                dma_rr(w["wq"], wts_d[p]["wq"][:, :])
                dma_rr(w["bq"], wts_d[p]["bq"][:, :])
                dma_rr(w["gvb"], wts_d[p]["gvb"][:, :])

            def load_pos(w, p):
                dma_rr(w["pos"][:, 0:1152], wts_d[p]["pos"][:, 0:1152])
                dma_rr(w["pos"][:, 1152:N], wts_d[p]["pos"][:, 1152:N])

            def load_big(dst, src):
                for q in range(4):
                    for k in range(KC):
                        sl = slice(k * N + q * NQ, k * N + (q + 1) * NQ)
                        dma_rr(dst[:, sl], src[:, sl])

            wt_e = wtiles("e")
            load_wv(wt_e, "e")           # first: V-proj is the first consumer
            load_big(xs_bf, xbf_d)
            load_small(wt_e, "e")
            load_pos(wt_e, "e")
            load_big(tot_bf, tbf_d)
            wt_d = wtiles("d")
            load_small(wt_d, "d")
            load_wv(wt_d, "d")
            load_pos(wt_d, "d")

            def proj_v(vt, wv, src, gamma, mi_range):
                for mi in mi_range:
                    vp = pl["pp_e"].tile([P, C], F32, tag="e", name=f"vp{mi}")
                    for k in range(KC):
                        nc.tensor.matmul(
                            vp,
                            src[:, k * N + mi * P : k * N + (mi + 1) * P],
                            wv[:, k * C : (k + 1) * C],
                            start=(k == 0),
                            stop=(k == KC - 1),
                        )
                    nc.vector.tensor_scalar_mul(
                        vt[:, mi * C : (mi + 1) * C], vp, float(gamma)
                    )

            def proj_k(kp, wk, pos, src, tiles):
                for n0, nw in tiles:
                    pp = pl["pp_e"].tile([P, 512], F32, tag="e", name="kpp")
                    for k in range(KC):
                        nc.tensor.matmul(
                            pp[:, :nw],
                            wk[:, k * P : (k + 1) * P],
                            src[:, k * N + n0 : k * N + n0 + nw],
                            start=(k == 0),
                            stop=(k == KC - 1),
                        )
                    nc.vector.tensor_add(
                        kp[:, n0 : n0 + nw], pp[:, :nw], pos[:, n0 : n0 + nw]
                    )

            def proj_q(qt, wq, bq, src):
                for n0, nw in GROUPS:
                    pp = pl["pp_e"].tile([P, 512], F32, tag="e", name="qpp")
                    for k in range(KC):
                        nc.tensor.matmul(
                            pp[:, :nw],
                            wq[:, k * P : (k + 1) * P],
                            src[:, k * N + n0 : k * N + n0 + nw],
                            start=(k == 0),
                            stop=(k == KC - 1),
                        )
                    nc.scalar.activation(
                        qt[:, n0 : n0 + nw], pp[:, :nw], AF.Identity, bias=bq
                    )

            def attn(kp, q, vt, wt, mode, post_group=None):
                """mode "enc": writes xenc_bf; "dec": DMAs out_cn."""
                enc = mode == "enc"
                for gi, (n0, gw) in enumerate(GROUPS):
                    exp_sb = pl["expe"].tile(
                        [P, NM * 512], BF16, tag="expe", name=f"exp_{mode}{gi}"
                    )
                    s_ps = pl["pp_s"].tile([P, 512], F32, tag="s", name="s_ps")
                    ops = [
                        pl["pp_out"].tile([P, 512], F32, tag="out", name=f"o{kc}")
                        for kc in range(KC)
                    ]
                    res_t = []
                    if not enc:
                        for kc in range(KC):
                            rt = pl["stream"].tile(
                                [P, 512], F32, tag="res", name=f"res{kc}"
                            )
                            nc.gpsimd.dma_start(
                                out=rt[:, :gw],
                                in_=xf_d[kc * P : (kc + 1) * P, n0 : n0 + gw],
                            )
                            res_t.append(rt)
                    for pr in range(0, NM, 2):
                        ea = pl["pp_e"].tile([P, 512], F32, tag="e", name="ea")
                        eb = pl["pp_e"].tile([P, 512], F32, tag="e", name="eb")
                        nc.tensor.matmul(
                            ea[:, :gw],
                            kp[0:C8, pr * P : (pr + 1) * P],
                            q[0:C8, n0 : n0 + gw],
                            start=True,
                            stop=True,
                        )
                        nc.tensor.matmul(
                            eb[:, :gw],
                            kp[C8:P, (pr + 1) * P : (pr + 2) * P],
                            q[C8:P, n0 : n0 + gw],
                            start=True,
                            stop=True,
                        )
                        sl0 = pr * 512
                        sl1 = (pr + 1) * 512
                        nc.scalar.activation(
                            exp_sb[:, sl0 : sl0 + gw], ea[:, :gw], AF.Exp
                        )
                        nc.scalar.activation(
                            exp_sb[:, sl1 : sl1 + gw], eb[:, :gw], AF.Exp
                        )
                        nc.tensor.matmul(
                            s_ps[:, :gw],
                            ones,
                            exp_sb[:, sl0 : sl0 + gw],
                            start=(pr == 0),
                            stop=False,
                        )
                        nc.tensor.matmul(
                            s_ps[:, :gw],
                            ones,
                            exp_sb[:, sl1 : sl1 + gw],
                            start=False,
                            stop=(pr == NM - 2),
                        )
                        for kc in range(KC):
                            nc.tensor.matmul(
                                ops[kc][:, :gw],
                                vt[:, (pr * KC + kc) * P : (pr * KC + kc + 1) * P],
                                exp_sb[:, sl0 : sl0 + gw],
                                start=(pr == 0),
                                stop=False,
                            )
                            nc.tensor.matmul(
                                ops[kc][:, :gw],
                                vt[:, ((pr + 1) * KC + kc) * P : ((pr + 1) * KC + kc + 1) * P],
                                exp_sb[:, sl1 : sl1 + gw],
                                start=False,
                                stop=(pr == NM - 2),
                            )
                    fb = pl["fbc"].tile([P, 512], F32, tag="fbc", name="fbc")
                    nc.vector.reciprocal_approx_fast(fb[:, :gw], s_ps[:, :gw])
                    for kc in range(KC):
                        t1 = pl["osb"].tile([P, 512], F32, tag="osb", name=f"t1_{kc}")
                        nc.vector.tensor_mul(
                            t1[:, :gw], ops[kc][:, :gw], fb[:, :gw]
                        )
                        if enc:
                            nc.vector.scalar_tensor_tensor(
                                out=xenc_bf[:, kc * N + n0 : kc * N + n0 + gw],
                                in0=t1[:, :gw],
                                scalar=wt["gvb"][:, kc : kc + 1],
                                in1=xs_bf[:, kc * N + n0 : kc * N + n0 + gw],
                                op0=OP.add,
                                op1=OP.add,
                            )
                        else:
                            ro = pl["osb"].tile(
                                [P, 512], F32, tag="osb", name=f"ro_{kc}"
                            )
                            nc.vector.scalar_tensor_tensor(
                                out=ro[:, :gw],
                                in0=t1[:, :gw],
                                scalar=wt["gvb"][:, kc : kc + 1],
                                in1=res_t[kc][:, :gw],
                                op0=OP.add,
                                op1=OP.add,
                            )
                            nc.sync.dma_start(
                                out=out_d[kc * P : (kc + 1) * P, n0 : n0 + gw],
                                in_=ro[:, :gw],
                            )
                    if post_group is not None and gi >= 1:
                        post_group(gi - 1)
                if post_group is not None:
                    post_group(len(GROUPS) - 1)

            # ---- encoder projections ----
            vt_e = pl["vt"].tile([P, NM * C], BF16, tag="vt", name="vt_e")
            kp_e = pl["kq"].tile([P, N], BF16, tag="kp", name="kp_e")
            q_e = pl["kq"].tile([P, N], BF16, tag="q", name="q_e")
            proj_v(vt_e, wt_e["wv"], xs_bf, gamma_e, range(NM))
            proj_k(kp_e, wt_e["wk"], wt_e["pos"], xs_bf, GROUPS)

            vt_d = pl["vt"].tile([P, NM * C], BF16, tag="vt", name="vt_d")
            kp_d = pl["kq"].tile([P, N], BF16, tag="kp", name="kp_d")
            q_d = pl["kq"].tile([P, N], BF16, tag="q", name="q_d")
            proj_q(q_d, wt_d["wq"], wt_d["bq"], xs_bf)
            proj_q(q_e, wt_e["wq"], wt_e["bq"], tot_bf)

            def enc_post(gi):
                n0, gw = GROUPS[gi]
                proj_k(kp_d, wt_d["wk"], wt_d["pos"], xenc_bf, [(n0, gw)])
                proj_v(
                    vt_d, wt_d["wv"], xenc_bf, gamma_d,
                    range(n0 // P, (n0 + gw) // P),
                )

            attn(kp_e, q_e, vt_e, wt_e, "enc", post_group=enc_post)
            attn(kp_d, q_d, vt_d, wt_d, "dec")

    nc.compile()
    return nc


def kernel(**inputs):
    import ml_dtypes

    def to_bf(a):
        return np.ascontiguousarray(a).astype(ml_dtypes.bfloat16)

    x = np.asarray(inputs["x"], np.float32)
    total = np.asarray(inputs["total"], np.float32)

    def pack_cn(a_cn):
        out = np.empty((P, KC * a_cn.shape[1]), a_cn.dtype)
        M = a_cn.shape[1]
        for k in range(KC):
            out[:, k * M : (k + 1) * M] = a_cn[k * P : (k + 1) * P]
        return out

    def prep(pfx):
        Wq = np.asarray(inputs[f"{pfx}_Wq"], np.float32)
        bq = np.asarray(inputs[f"{pfx}_bq"], np.float32)
        Wk = np.asarray(inputs[f"{pfx}_Wk"], np.float32)
        bk = np.asarray(inputs[f"{pfx}_bk"], np.float32)
        Wv = np.asarray(inputs[f"{pfx}_Wv"], np.float32)
        bv = np.asarray(inputs[f"{pfx}_bv"], np.float32)
        ht = np.asarray(inputs[f"{pfx}_ht"], np.float32)
        wtt = np.asarray(inputs[f"{pfx}_wt"], np.float32)
        gamma = float(np.asarray(inputs[f"{pfx}_gamma"], np.float32).reshape(-1)[0])
        pos = (ht + wtt).reshape(C8, N) + bk[:, None]

        def dup_pack(wT):
            # [C, C8] -> [128, KC*128]: chunk k cols = [wT_k | wT_k]
            out = np.empty((P, KC * P), np.float32)
            for k in range(KC):
                blk = wT[k * P : (k + 1) * P]            # [128, 64]
                out[:, k * P : k * P + C8] = blk
                out[:, k * P + C8 : (k + 1) * P] = blk
            return out

        pos_dup = np.concatenate([pos, pos], axis=0)      # [128, N]
        bq_dup = np.concatenate([bq, bq]).reshape(P, 1)
        gvb = np.ascontiguousarray((gamma * bv).reshape(KC, P).T)  # [128, KC]
        return {
            "wk": to_bf(dup_pack(np.ascontiguousarray(Wk.T))),
            "wq": to_bf(dup_pack(np.ascontiguousarray(Wq.T))),
            "wv": to_bf(pack_cn(np.ascontiguousarray(Wv.T))),
            "pos": to_bf(pos_dup),
            "bq": np.ascontiguousarray(bq_dup),
            "gvb": gvb,
            "gamma": gamma,
        }

    pe, pd = prep("enc"), prep("dec")
    nc = build_bass(pe["gamma"], pd["gamma"])

    in_maps = []
    for b in range(B):
        x_cn = np.ascontiguousarray(x[b].reshape(C, N))
        tot_cn = np.ascontiguousarray(total[b].reshape(C, N))
        m = {
            "x_bf": to_bf(pack_cn(x_cn)),
            "tot_bf": to_bf(pack_cn(tot_cn)),
            "x_cn": x_cn,
        }
        for p, w in (("e", pe), ("d", pd)):
            for key in ("wk", "wq", "wv", "pos", "bq", "gvb"):
                m[f"{key}_{p}"] = w[key]
        in_maps.append(m)

    res = run_bass_kernel_spmd(nc, in_maps, core_ids=list(range(B)))
    out = np.stack(
        [res.results[b]["out_cn"].reshape(C, H, W) for b in range(B)], axis=0
    )
    return out.astype(np.float32)


if __name__ == "__main__":
    import reference

    ins = {k: np.asarray(v) for k, v in reference.setup_inputs().items()}
    got = kernel(**ins)
    exp = np.asarray(reference.reference(**ins))
    err = np.abs(got - exp).max() / (np.abs(exp).max() + 1e-30)
    print("abs-rel err:", err)


# revision 8
# speedup vs baseline: 1.1989x; 1.0469x over previous
"""Bass/Trainium2 kernel for nn_Attention_47622597378289.

Two chained attention blocks (encoder, decoder) over [B=8, C=512, H=W=48].
Data-parallel over batch: core i handles batch item i (B == n_cores == 8).

v3 design:
  - All matmul operands bf16 (host-cast): no fp32 HIGH/LOW_HIGH PE modes,
    FWL-fast weight loads throughout.
  - Out matmul computed TRANSPOSED: out[c,n] = sum_m vt[m,c].T @ exp[m,n].
    No PE transposes, x_enc lands directly in [c,n] layout for the decoder,
    and Out consumes exp chunks as they are produced (per-mi pipeline:
    ScalarE exp hides under PE work).
  - E matmuls row-packed 2x via tile_position (K=C8=64) with duplicated
    kp/q partition halves.
  - S via all-ones [128,128] lhsT -> arrives pre-broadcast [128,gw];
    1/S with DVE reciprocal_approx_fast; gamma folded into vt, gamma*bv
    folded as per-partition scalar in the residual STT.
  - Startup DMA round-robined over 3 HWDGE rings (~118GB/s each), wv
    first (V-proj is the first consumer), pos in bf16.
  - Decoder projections emitted one group LATE so the DVE/GpSimd residual
    ops for x_enc finish before PE needs them; residual STT on GpSimd.
"""

import numpy as np

import concourse.bass as bass
import concourse.bacc as bacc
import concourse.mybir as mybir
from concourse.bass_utils import run_bass_kernel_spmd
from concourse.tile import TileContext

F32 = mybir.dt.float32
BF16 = mybir.dt.bfloat16
AF = mybir.ActivationFunctionType
OP = mybir.AluOpType

B, C, H, W = 8, 512, 48, 48
C8 = C // 8          # 64
N = H * W            # 2304
P = 128
KC = C // P          # 4 c-chunks
NM = N // P          # 18 m-chunks
GROUPS = [(0, 512), (512, 512), (1024, 512), (1536, 512), (2048, 256)]
NQ = N // 4          # 576-col load quarters


def build_bass(gamma_e, gamma_d):
    nc = bacc.Bacc("TRN2", target_bir_lowering=False, debug=False)

    xbf_d = nc.dram_tensor("x_bf", [P, KC * N], BF16, kind="ExternalInput")
    tbf_d = nc.dram_tensor("tot_bf", [P, KC * N], BF16, kind="ExternalInput")
    xf_d = nc.dram_tensor("x_cn", [C, N], F32, kind="ExternalInput")
    wts_d = {}
    for p in ("e", "d"):
        wts_d[p] = {
            "wk": nc.dram_tensor(f"wk_{p}", [P, KC * P], BF16, kind="ExternalInput"),
            "wq": nc.dram_tensor(f"wq_{p}", [P, KC * P], BF16, kind="ExternalInput"),
            "wv": nc.dram_tensor(f"wv_{p}", [P, KC * C], BF16, kind="ExternalInput"),
            "pos": nc.dram_tensor(f"pos_{p}", [P, N], BF16, kind="ExternalInput"),
            "bq": nc.dram_tensor(f"bq_{p}", [P, 1], F32, kind="ExternalInput"),
            "gvb": nc.dram_tensor(f"gvb_{p}", [P, KC], F32, kind="ExternalInput"),
        }
    out_d = nc.dram_tensor("out_cn", [C, N], F32, kind="ExternalOutput")

    with TileContext(nc) as tc:
        import contextlib

        with contextlib.ExitStack() as ctx:
            pl = {
                "persist": ctx.enter_context(tc.tile_pool(name="persist", bufs=1)),
                "wpool": ctx.enter_context(tc.tile_pool(name="wpool", bufs=1)),
                "kq": ctx.enter_context(tc.tile_pool(name="kq", bufs=2)),
                "vt": ctx.enter_context(tc.tile_pool(name="vt", bufs=2)),
                "expe": ctx.enter_context(tc.tile_pool(name="expe", bufs=2)),
                "fbc": ctx.enter_context(tc.tile_pool(name="fbc", bufs=2)),
                "osb": ctx.enter_context(tc.tile_pool(name="osb", bufs=8)),
                "stream": ctx.enter_context(tc.tile_pool(name="stream", bufs=4)),
                "pp_e": ctx.enter_context(
                    tc.tile_pool(name="pp_e", bufs=3, space="PSUM")
                ),
                "pp_out": ctx.enter_context(
                    tc.tile_pool(name="pp_out", bufs=4, space="PSUM")
                ),
                "pp_s": ctx.enter_context(
                    tc.tile_pool(name="pp_s", bufs=1, space="PSUM")
                ),
            }
            wpool = pl["wpool"]
            persist = pl["persist"]

            ones = wpool.tile([P, P], BF16, tag="ones")
            nc.vector.memset(ones, 1.0)

            xs_bf = persist.tile([P, KC * N], BF16, tag="xs_bf")
            tot_bf = persist.tile([P, KC * N], BF16, tag="tot_bf")
            xenc_bf = persist.tile([P, KC * N], BF16, tag="xenc_bf")

            rings = [nc.sync, nc.scalar, nc.gpsimd]
            ring_i = [0]

            def dma_rr(out, in_):
                rings[ring_i[0] % 3].dma_start(out=out, in_=in_)
                ring_i[0] += 1

            def wtiles(p):
                return {
                    "wk": wpool.tile([P, KC * P], BF16, tag=f"wk{p}", name=f"wk_{p}"),
                    "wq": wpool.tile([P, KC * P], BF16, tag=f"wq{p}", name=f"wq_{p}"),
                    "wv": wpool.tile([P, KC * C], BF16, tag=f"wv{p}", name=f"wv_{p}"),
                    "pos": wpool.tile([P, N], BF16, tag=f"pos{p}", name=f"pos_{p}"),
                    "bq": wpool.tile([P, 1], F32, tag=f"bq{p}", name=f"bq_{p}"),
                    "gvb": wpool.tile([P, KC], F32, tag=f"gvb{p}", name=f"gvb_{p}"),
                }

            def load_wv(w, p):
                for c0, c1 in ((0, 768), (768, 1536), (1536, KC * C)):
                    dma_rr(w["wv"][:, c0:c1], wts_d[p]["wv"][:, c0:c1])

            def load_small(w, p):
                dma_rr(w["wk"], wts_d[p]["wk"][:, :])
                dma_rr(w["wq"], wts_d[p]["wq"][:, :])
                dma_rr(w["bq"], wts_d[p]["bq"][:, :])
                dma_rr(w["gvb"], wts_d[p]["gvb"][:, :])

            def load_pos(w, p):
                dma_rr(w["pos"][:, 0:1152], wts_d[p]["pos"][:, 0:1152])
                dma_rr(w["pos"][:, 1152:N], wts_d[p]["pos"][:, 1152:N])

            def load_big(dst, src):
                for q in range(4):
                    for k in range(KC):
                        sl = slice(k * N + q * NQ, k * N + (q + 1) * NQ)
                        dma_rr(dst[:, sl], src[:, sl])

            wt_e = wtiles("e")
            load_wv(wt_e, "e")           # first: V-proj is the first consumer
            load_big(xs_bf, xbf_d)
            load_small(wt_e, "e")
            load_pos(wt_e, "e")
            load_big(tot_bf, tbf_d)
            wt_d = wtiles("d")
            load_small(wt_d, "d")
            load_wv(wt_d, "d")
            load_pos(wt_d, "d")

            def proj_v(vt, wv, src, gamma, mi_range):
                for mi in mi_range:
                    vp = pl["pp_e"].tile([P, C], F32, tag="e", name=f"vp{mi}")
                    for k in range(KC):
                        nc.tensor.matmul(
                            vp,
                            src[:, k * N + mi * P : k * N + (mi + 1) * P],
                            wv[:, k * C : (k + 1) * C],
                            start=(k == 0),
                            stop=(k == KC - 1),
                        )
                    nc.vector.tensor_scalar_mul(
                        vt[:, mi * C : (mi + 1) * C], vp, float(gamma)
                    )

            def proj_k(kp, wk, pos, src, tiles):
                for n0, nw in tiles:
                    pp = pl["pp_e"].tile([P, 512], F32, tag="e", name="kpp")
                    for k in range(KC):
                        nc.tensor.matmul(
                            pp[:, :nw],
                            wk[:, k * P : (k + 1) * P],
                            src[:, k * N + n0 : k * N + n0 + nw],
                            start=(k == 0),
                            stop=(k == KC - 1),
                        )
                    nc.vector.tensor_add(
                        kp[:, n0 : n0 + nw], pp[:, :nw], pos[:, n0 : n0 + nw]
                    )

            def proj_q(qt, wq, bq, src):
                for n0, nw in GROUPS:
                    pp = pl["pp_e"].tile([P, 512], F32, tag="e", name="qpp")
                    for k in range(KC):
                        nc.tensor.matmul(
                            pp[:, :nw],
                            wq[:, k * P : (k + 1) * P],
                            src[:, k * N + n0 : k * N + n0 + nw],
                            start=(k == 0),
                            stop=(k == KC - 1),
                        )
                    nc.scalar.activation(
                        qt[:, n0 : n0 + nw], pp[:, :nw], AF.Identity, bias=bq
                    )

            def attn(kp, q, vt, wt, mode, post_group=None):
                """mode "enc": writes xenc_bf; "dec": DMAs out_cn."""
                enc = mode == "enc"
                for gi, (n0, gw) in enumerate(GROUPS):
                    exp_sb = pl["expe"].tile(
                        [P, NM * 512], BF16, tag="expe", name=f"exp_{mode}{gi}"
                    )
                    s_ps = pl["pp_s"].tile([P, 512], F32, tag="s", name="s_ps")
                    ops = [
                        pl["pp_out"].tile([P, 512], F32, tag="out", name=f"o{kc}")
                        for kc in range(KC)
                    ]
                    res_t = []
                    if not enc:
                        for kc in range(KC):
                            rt = pl["stream"].tile(
                                [P, 512], F32, tag="res", name=f"res{kc}"
                            )
                            nc.gpsimd.dma_start(
                                out=rt[:, :gw],
                                in_=xf_d[kc * P : (kc + 1) * P, n0 : n0 + gw],
                            )
                            res_t.append(rt)
                    for pr in range(0, NM, 2):
                        ea = pl["pp_e"].tile([P, 512], F32, tag="e", name="ea")
                        eb = pl["pp_e"].tile([P, 512], F32, tag="e", name="eb")
                        nc.tensor.matmul(
                            ea[:, :gw],
                            kp[0:C8, pr * P : (pr + 1) * P],
                            q[0:C8, n0 : n0 + gw],
                            start=True,
                            stop=True,
                        )
                        nc.tensor.matmul(
                            eb[:, :gw],
                            kp[C8:P, (pr + 1) * P : (pr + 2) * P],
                            q[C8:P, n0 : n0 + gw],
                            start=True,
                            stop=True,
                        )
                        sl0 = pr * 512
                        sl1 = (pr + 1) * 512
                        nc.scalar.activation(
                            exp_sb[:, sl0 : sl0 + gw], ea[:, :gw], AF.Exp
                        )
                        nc.scalar.activation(
                            exp_sb[:, sl1 : sl1 + gw], eb[:, :gw], AF.Exp
                        )
                        nc.tensor.matmul(
                            s_ps[:, :gw],
                            ones,
                            exp_sb[:, sl0 : sl0 + gw],
                            start=(pr == 0),
                            stop=False,
                        )
                        nc.tensor.matmul(
                            s_ps[:, :gw],
                            ones,
                            exp_sb[:, sl1 : sl1 + gw],
                            start=False,
                            stop=(pr == NM - 2),
                        )
                        for kc in range(KC):
                            nc.tensor.matmul(
                                ops[kc][:, :gw],
                                vt[:, (pr * KC + kc) * P : (pr * KC + kc + 1) * P],
                                exp_sb[:, sl0 : sl0 + gw],
                                start=(pr == 0),
                                stop=False,
                            )
                            nc.tensor.matmul(
                                ops[kc][:, :gw],
                                vt[:, ((pr + 1) * KC + kc) * P : ((pr + 1) * KC + kc + 1) * P],
                                exp_sb[:, sl1 : sl1 + gw],
                                start=False,
                                stop=(pr == NM - 2),
                            )
                    fb = pl["fbc"].tile([P, 512], F32, tag="fbc", name="fbc")
                    nc.vector.reciprocal_approx_fast(fb[:, :gw], s_ps[:, :gw])
                    t1s = []
                    for kc in range(KC):
                        t1 = pl["osb"].tile([P, 512], F32, tag="osb", name=f"t1_{kc}")
                        nc.vector.tensor_mul(
                            t1[:, :gw], ops[kc][:, :gw], fb[:, :gw]
                        )
                        t1s.append(t1)
                    # dec projections for the PREVIOUS group, emitted before the
                    # xenc STT writes below: the PE picks them up with no DVE
                    # dependency, bridging the group boundary.
                    if post_group is not None and gi >= 1:
                        post_group(gi - 1)
                    for kc in range(KC):
                        if enc:
                            nc.vector.scalar_tensor_tensor(
                                out=xenc_bf[:, kc * N + n0 : kc * N + n0 + gw],
                                in0=t1s[kc][:, :gw],
                                scalar=wt["gvb"][:, kc : kc + 1],
                                in1=xs_bf[:, kc * N + n0 : kc * N + n0 + gw],
                                op0=OP.add,
                                op1=OP.add,
                            )
                        else:
                            ro = pl["osb"].tile(
                                [P, 512], F32, tag="osb", name=f"ro_{kc}"
                            )
                            nc.vector.scalar_tensor_tensor(
                                out=ro[:, :gw],
                                in0=t1s[kc][:, :gw],
                                scalar=wt["gvb"][:, kc : kc + 1],
                                in1=res_t[kc][:, :gw],
                                op0=OP.add,
                                op1=OP.add,
                            )
                            nc.sync.dma_start(
                                out=out_d[kc * P : (kc + 1) * P, n0 : n0 + gw],
                                in_=ro[:, :gw],
                            )
                if post_group is not None:
                    post_group(len(GROUPS) - 1)

            # ---- encoder projections ----
            vt_e = pl["vt"].tile([P, NM * C], BF16, tag="vt", name="vt_e")
            kp_e = pl["kq"].tile([P, N], BF16, tag="kp", name="kp_e")
            q_e = pl["kq"].tile([P, N], BF16, tag="q", name="q_e")
            proj_v(vt_e, wt_e["wv"], xs_bf, gamma_e, range(NM))
            proj_k(kp_e, wt_e["wk"], wt_e["pos"], xs_bf, GROUPS)

            vt_d = pl["vt"].tile([P, NM * C], BF16, tag="vt", name="vt_d")
            kp_d = pl["kq"].tile([P, N], BF16, tag="kp", name="kp_d")
            q_d = pl["kq"].tile([P, N], BF16, tag="q", name="q_d")
            proj_q(q_d, wt_d["wq"], wt_d["bq"], xs_bf)
            proj_q(q_e, wt_e["wq"], wt_e["bq"], tot_bf)

            def enc_post(gi):
                n0, gw = GROUPS[gi]
                proj_k(kp_d, wt_d["wk"], wt_d["pos"], xenc_bf, [(n0, gw)])
                proj_v(
                    vt_d, wt_d["wv"], xenc_bf, gamma_d,
                    range(n0 // P, (n0 + gw) // P),
                )

            attn(kp_e, q_e, vt_e, wt_e, "enc", post_group=enc_post)
            attn(kp_d, q_d, vt_d, wt_d, "dec")

    nc.compile()
    return nc


def kernel(**inputs):
    import ml_dtypes

    def to_bf(a):
        return np.ascontiguousarray(a).astype(ml_dtypes.bfloat16)

    x = np.asarray(inputs["x"], np.float32)
    total = np.asarray(inputs["total"], np.float32)

    def pack_cn(a_cn):
        out = np.empty((P, KC * a_cn.shape[1]), a_cn.dtype)
        M = a_cn.shape[1]
        for k in range(KC):
            out[:, k * M : (k + 1) * M] = a_cn[k * P : (k + 1) * P]
        return out

    def prep(pfx):
        Wq = np.asarray(inputs[f"{pfx}_Wq"], np.float32)
        bq = np.asarray(inputs[f"{pfx}_bq"], np.float32)
        Wk = np.asarray(inputs[f"{pfx}_Wk"], np.float32)
        bk = np.asarray(inputs[f"{pfx}_bk"], np.float32)
        Wv = np.asarray(inputs[f"{pfx}_Wv"], np.float32)
        bv = np.asarray(inputs[f"{pfx}_bv"], np.float32)
        ht = np.asarray(inputs[f"{pfx}_ht"], np.float32)
        wtt = np.asarray(inputs[f"{pfx}_wt"], np.float32)
        gamma = float(np.asarray(inputs[f"{pfx}_gamma"], np.float32).reshape(-1)[0])
        pos = (ht + wtt).reshape(C8, N) + bk[:, None]

        def dup_pack(wT):
            # [C, C8] -> [128, KC*128]: chunk k cols = [wT_k | wT_k]
            out = np.empty((P, KC * P), np.float32)
            for k in range(KC):
                blk = wT[k * P : (k + 1) * P]            # [128, 64]
                out[:, k * P : k * P + C8] = blk
                out[:, k * P + C8 : (k + 1) * P] = blk
            return out

        pos_dup = np.concatenate([pos, pos], axis=0)      # [128, N]
        bq_dup = np.concatenate([bq, bq]).reshape(P, 1)
        gvb = np.ascontiguousarray((gamma * bv).reshape(KC, P).T)  # [128, KC]
        return {
            "wk": to_bf(dup_pack(np.ascontiguousarray(Wk.T))),
            "wq": to_bf(dup_pack(np.ascontiguousarray(Wq.T))),
            "wv": to_bf(pack_cn(np.ascontiguousarray(Wv.T))),
            "pos": to_bf(pos_dup),
            "bq": np.ascontiguousarray(bq_dup),
            "gvb": gvb,
            "gamma": gamma,
        }

    pe, pd = prep("enc"), prep("dec")
    nc = build_bass(pe["gamma"], pd["gamma"])

    in_maps = []
    for b in range(B):
        x_cn = np.ascontiguousarray(x[b].reshape(C, N))
        tot_cn = np.ascontiguousarray(total[b].reshape(C, N))
        m = {
            "x_bf": to_bf(pack_cn(x_cn)),
            "tot_bf": to_bf(pack_cn(tot_cn)),
            "x_cn": x_cn,
        }
        for p, w in (("e", pe), ("d", pd)):
            for key in ("wk", "wq", "wv", "pos", "bq", "gvb"):
                m[f"{key}_{p}"] = w[key]
        in_maps.append(m)

    res = run_bass_kernel_spmd(nc, in_maps, core_ids=list(range(B)))
    out = np.stack(
        [res.results[b]["out_cn"].reshape(C, H, W) for b in range(B)], axis=0
    )
    return out.astype(np.float32)


if __name__ == "__main__":
    import reference

    ins = {k: np.asarray(v) for k, v in reference.setup_inputs().items()}
    got = kernel(**ins)
    exp = np.asarray(reference.reference(**ins))
    err = np.abs(got - exp).max() / (np.abs(exp).max() + 1e-30)
    print("abs-rel err:", err)


# revision 9
# speedup vs baseline: 1.3277x; 1.1074x over previous
"""Bass/Trainium2 kernel for nn_Attention_47622597378289.

Two chained attention blocks (encoder, decoder) over [B=8, C=512, H=W=48].
Data-parallel over batch: core i handles batch item i (B == n_cores == 8).

v3 design:
  - All matmul operands bf16 (host-cast): no fp32 HIGH/LOW_HIGH PE modes,
    FWL-fast weight loads throughout.
  - Out matmul computed TRANSPOSED: out[c,n] = sum_m vt[m,c].T @ exp[m,n].
    No PE transposes, x_enc lands directly in [c,n] layout for the decoder,
    and Out consumes exp chunks as they are produced (per-mi pipeline:
    ScalarE exp hides under PE work).
  - E matmuls row-packed 2x via tile_position (K=C8=64) with duplicated
    kp/q partition halves.
  - S via all-ones [128,128] lhsT -> arrives pre-broadcast [128,gw];
    1/S with DVE reciprocal_approx_fast; gamma folded into vt, gamma*bv
    folded as per-partition scalar in the residual STT.
  - Startup DMA round-robined over 3 HWDGE rings (~118GB/s each), wv
    first (V-proj is the first consumer), pos in bf16.
  - Decoder projections emitted one group LATE so the DVE/GpSimd residual
    ops for x_enc finish before PE needs them; residual STT on GpSimd.
"""

import numpy as np

import concourse.bass as bass
import concourse.bacc as bacc
import concourse.mybir as mybir
from concourse.bass_utils import run_bass_kernel_spmd
from concourse.tile import TileContext

F32 = mybir.dt.float32
BF16 = mybir.dt.bfloat16
AF = mybir.ActivationFunctionType
OP = mybir.AluOpType

B, C, H, W = 8, 512, 48, 48
C8 = C // 8          # 64
N = H * W            # 2304
P = 128
KC = C // P          # 4 c-chunks
NM = N // P          # 18 m-chunks
GROUPS = [(0, 512), (512, 512), (1024, 512), (1536, 512), (2048, 256)]
NQ = N // 4          # 576-col load quarters


def build_bass(gamma_e, gamma_d):
    nc = bacc.Bacc("TRN2", target_bir_lowering=False, debug=False)

    xbf_d = nc.dram_tensor("x_bf", [P, KC * N], BF16, kind="ExternalInput")
    tbf_d = nc.dram_tensor("tot_bf", [P, KC * N], BF16, kind="ExternalInput")
    xf_d = nc.dram_tensor("x_cn", [C, N], F32, kind="ExternalInput")
    wts_d = {}
    for p in ("e", "d"):
        wts_d[p] = {
            "wk": nc.dram_tensor(f"wk_{p}", [P, KC * P], BF16, kind="ExternalInput"),
            "wq": nc.dram_tensor(f"wq_{p}", [P, KC * P], BF16, kind="ExternalInput"),
            "wv": nc.dram_tensor(f"wv_{p}", [P, KC * C], BF16, kind="ExternalInput"),
            "pos": nc.dram_tensor(f"pos_{p}", [P, N], BF16, kind="ExternalInput"),
            "bq": nc.dram_tensor(f"bq_{p}", [P, 1], F32, kind="ExternalInput"),
            "gvb": nc.dram_tensor(f"gvb_{p}", [P, KC], F32, kind="ExternalInput"),
        }
    out_d = nc.dram_tensor("out_cn", [C, N], F32, kind="ExternalOutput")

    with TileContext(nc) as tc:
        import contextlib

        with contextlib.ExitStack() as ctx:
            pl = {
                "persist": ctx.enter_context(tc.tile_pool(name="persist", bufs=1)),
                "wpool": ctx.enter_context(tc.tile_pool(name="wpool", bufs=1)),
                "kq": ctx.enter_context(tc.tile_pool(name="kq", bufs=2)),
                "vt": ctx.enter_context(tc.tile_pool(name="vt", bufs=2)),
                "expe": ctx.enter_context(tc.tile_pool(name="expe", bufs=2)),
                "fbc": ctx.enter_context(tc.tile_pool(name="fbc", bufs=2)),
                "osb": ctx.enter_context(tc.tile_pool(name="osb", bufs=8)),
                "stream": ctx.enter_context(tc.tile_pool(name="stream", bufs=4)),
                "pp_e": ctx.enter_context(
                    tc.tile_pool(name="pp_e", bufs=3, space="PSUM")
                ),
                "pp_out": ctx.enter_context(
                    tc.tile_pool(name="pp_out", bufs=4, space="PSUM")
                ),
                "pp_s": ctx.enter_context(
                    tc.tile_pool(name="pp_s", bufs=1, space="PSUM")
                ),
            }
            wpool = pl["wpool"]
            persist = pl["persist"]

            ones = wpool.tile([P, P], BF16, tag="ones")
            nc.vector.memset(ones, 1.0)

            xs_bf = persist.tile([P, KC * N], BF16, tag="xs_bf")
            tot_bf = persist.tile([P, KC * N], BF16, tag="tot_bf")
            xenc_bf = persist.tile([P, KC * N], BF16, tag="xenc_bf")

            rings = [nc.sync, nc.scalar, nc.gpsimd]
            ring_i = [0]

            def dma_rr(out, in_):
                rings[ring_i[0] % 3].dma_start(out=out, in_=in_)
                ring_i[0] += 1

            def wtiles(p):
                return {
                    "wk": wpool.tile([P, KC * P], BF16, tag=f"wk{p}", name=f"wk_{p}"),
                    "wq": wpool.tile([P, KC * P], BF16, tag=f"wq{p}", name=f"wq_{p}"),
                    "wv": wpool.tile([P, KC * C], BF16, tag=f"wv{p}", name=f"wv_{p}"),
                    "pos": wpool.tile([P, N], BF16, tag=f"pos{p}", name=f"pos_{p}"),
                    "bq": wpool.tile([P, 1], F32, tag=f"bq{p}", name=f"bq_{p}"),
                    "gvb": wpool.tile([P, KC], F32, tag=f"gvb{p}", name=f"gvb_{p}"),
                }

            def load_wv(w, p):
                for c0, c1 in ((0, 768), (768, 1536), (1536, KC * C)):
                    dma_rr(w["wv"][:, c0:c1], wts_d[p]["wv"][:, c0:c1])

            def load_small(w, p):
                dma_rr(w["wk"], wts_d[p]["wk"][:, :])
                dma_rr(w["wq"], wts_d[p]["wq"][:, :])
                dma_rr(w["bq"], wts_d[p]["bq"][:, :])
                dma_rr(w["gvb"], wts_d[p]["gvb"][:, :])

            def load_pos(w, p):
                dma_rr(w["pos"][:, 0:1152], wts_d[p]["pos"][:, 0:1152])
                dma_rr(w["pos"][:, 1152:N], wts_d[p]["pos"][:, 1152:N])

            def load_big(dst, src):
                for q in range(4):
                    for k in range(KC):
                        sl = slice(k * N + q * NQ, k * N + (q + 1) * NQ)
                        dma_rr(dst[:, sl], src[:, sl])

            wt_e = wtiles("e")
            load_wv(wt_e, "e")           # first: V-proj is the first consumer
            load_big(xs_bf, xbf_d)
            load_small(wt_e, "e")
            load_pos(wt_e, "e")
            load_big(tot_bf, tbf_d)
            wt_d = wtiles("d")
            load_small(wt_d, "d")
            load_wv(wt_d, "d")
            load_pos(wt_d, "d")

            def proj_v(vt, wv, src, gamma, mi_range):
                for mi in mi_range:
                    vp = pl["pp_e"].tile([P, C], F32, tag="e", name=f"vp{mi}")
                    for k in range(KC):
                        nc.tensor.matmul(
                            vp,
                            src[:, k * N + mi * P : k * N + (mi + 1) * P],
                            wv[:, k * C : (k + 1) * C],
                            start=(k == 0),
                            stop=(k == KC - 1),
                        )
                    nc.vector.tensor_scalar_mul(
                        vt[:, mi * C : (mi + 1) * C], vp, float(gamma)
                    )

            def proj_k(kp, wk, pos, src, tiles):
                for n0, nw in tiles:
                    pp = pl["pp_e"].tile([P, 512], F32, tag="e", name="kpp")
                    for k in range(KC):
                        nc.tensor.matmul(
                            pp[:, :nw],
                            wk[:, k * P : (k + 1) * P],
                            src[:, k * N + n0 : k * N + n0 + nw],
                            start=(k == 0),
                            stop=(k == KC - 1),
                        )
                    nc.vector.tensor_add(
                        kp[:, n0 : n0 + nw], pp[:, :nw], pos[:, n0 : n0 + nw]
                    )

            def proj_q(qt, wq, bq, src):
                for n0, nw in GROUPS:
                    pp = pl["pp_e"].tile([P, 512], F32, tag="e", name="qpp")
                    for k in range(KC):
                        nc.tensor.matmul(
                            pp[:, :nw],
                            wq[:, k * P : (k + 1) * P],
                            src[:, k * N + n0 : k * N + n0 + nw],
                            start=(k == 0),
                            stop=(k == KC - 1),
                        )
                    nc.scalar.activation(
                        qt[:, n0 : n0 + nw], pp[:, :nw], AF.Identity, bias=bq
                    )

            def attn(kp, q, vt, wt, mode, post_group=None):
                """mode "enc": writes xenc_bf; "dec": DMAs out_cn."""
                enc = mode == "enc"
                for gi, (n0, gw) in enumerate(GROUPS):
                    exp_sb = pl["expe"].tile(
                        [P, NM * 512], BF16, tag="expe", name=f"exp_{mode}{gi}"
                    )
                    s_ps = pl["pp_s"].tile([P, 512], F32, tag="s", name="s_ps")
                    ops = [
                        pl["pp_out"].tile([P, 512], F32, tag="out", name=f"o{kc}")
                        for kc in range(KC)
                    ]
                    res_t = []
                    if not enc:
                        for kc in range(KC):
                            rt = pl["stream"].tile(
                                [P, 512], F32, tag="res", name=f"res{kc}"
                            )
                            nc.gpsimd.dma_start(
                                out=rt[:, :gw],
                                in_=xf_d[kc * P : (kc + 1) * P, n0 : n0 + gw],
                            )
                            res_t.append(rt)
                    def epair(pr):
                        ea = pl["pp_e"].tile([P, 512], F32, tag="e", name="ea")
                        eb = pl["pp_e"].tile([P, 512], F32, tag="e", name="eb")
                        nc.tensor.matmul(
                            ea[:, :gw],
                            kp[0:C8, pr * P : (pr + 1) * P],
                            q[0:C8, n0 : n0 + gw],
                            start=True,
                            stop=True,
                        )
                        nc.tensor.matmul(
                            eb[:, :gw],
                            kp[C8:P, (pr + 1) * P : (pr + 2) * P],
                            q[C8:P, n0 : n0 + gw],
                            start=True,
                            stop=True,
                        )
                        nc.scalar.activation(
                            exp_sb[:, pr * 512 : pr * 512 + gw], ea[:, :gw], AF.Exp
                        )
                        nc.scalar.activation(
                            exp_sb[:, (pr + 1) * 512 : (pr + 1) * 512 + gw],
                            eb[:, :gw],
                            AF.Exp,
                        )

                    epair(0)
                    for pr in range(0, NM, 2):
                        # one-iteration lookahead: E/exp for the next pair are
                        # in flight while S/Out consume this pair.
                        if pr + 2 < NM:
                            epair(pr + 2)
                        sl0 = pr * 512
                        sl1 = (pr + 1) * 512
                        nc.tensor.matmul(
                            s_ps[:, :gw],
                            ones,
                            exp_sb[:, sl0 : sl0 + gw],
                            start=(pr == 0),
                            stop=False,
                        )
                        nc.tensor.matmul(
                            s_ps[:, :gw],
                            ones,
                            exp_sb[:, sl1 : sl1 + gw],
                            start=False,
                            stop=(pr == NM - 2),
                        )
                        for kc in range(KC):
                            nc.tensor.matmul(
                                ops[kc][:, :gw],
                                vt[:, (pr * KC + kc) * P : (pr * KC + kc + 1) * P],
                                exp_sb[:, sl0 : sl0 + gw],
                                start=(pr == 0),
                                stop=False,
                            )
                            nc.tensor.matmul(
                                ops[kc][:, :gw],
                                vt[:, ((pr + 1) * KC + kc) * P : ((pr + 1) * KC + kc + 1) * P],
                                exp_sb[:, sl1 : sl1 + gw],
                                start=False,
                                stop=(pr == NM - 2),
                            )
                    fb = pl["fbc"].tile([P, 512], F32, tag="fbc", name="fbc")
                    nc.vector.reciprocal_approx_fast(fb[:, :gw], s_ps[:, :gw])
                    t1s = []
                    for kc in range(KC):
                        t1 = pl["osb"].tile([P, 512], F32, tag="osb", name=f"t1_{kc}")
                        nc.vector.tensor_mul(
                            t1[:, :gw], ops[kc][:, :gw], fb[:, :gw]
                        )
                        t1s.append(t1)
                    # dec projections for the PREVIOUS group, emitted before the
                    # xenc STT writes below: the PE picks them up with no DVE
                    # dependency, bridging the group boundary.
                    if post_group is not None and gi >= 1:
                        post_group(gi - 1)
                    for kc in range(KC):
                        if enc:
                            nc.vector.scalar_tensor_tensor(
                                out=xenc_bf[:, kc * N + n0 : kc * N + n0 + gw],
                                in0=t1s[kc][:, :gw],
                                scalar=wt["gvb"][:, kc : kc + 1],
                                in1=xs_bf[:, kc * N + n0 : kc * N + n0 + gw],
                                op0=OP.add,
                                op1=OP.add,
                            )
                        else:
                            ro = pl["osb"].tile(
                                [P, 512], F32, tag="osb", name=f"ro_{kc}"
                            )
                            nc.vector.scalar_tensor_tensor(
                                out=ro[:, :gw],
                                in0=t1s[kc][:, :gw],
                                scalar=wt["gvb"][:, kc : kc + 1],
                                in1=res_t[kc][:, :gw],
                                op0=OP.add,
                                op1=OP.add,
                            )
                            nc.sync.dma_start(
                                out=out_d[kc * P : (kc + 1) * P, n0 : n0 + gw],
                                in_=ro[:, :gw],
                            )
                if post_group is not None:
                    post_group(len(GROUPS) - 1)

            # ---- encoder projections ----
            vt_e = pl["vt"].tile([P, NM * C], BF16, tag="vt", name="vt_e")
            kp_e = pl["kq"].tile([P, N], BF16, tag="kp", name="kp_e")
            q_e = pl["kq"].tile([P, N], BF16, tag="q", name="q_e")
            proj_v(vt_e, wt_e["wv"], xs_bf, gamma_e, range(NM))
            proj_k(kp_e, wt_e["wk"], wt_e["pos"], xs_bf, GROUPS)

            vt_d = pl["vt"].tile([P, NM * C], BF16, tag="vt", name="vt_d")
            kp_d = pl["kq"].tile([P, N], BF16, tag="kp", name="kp_d")
            q_d = pl["kq"].tile([P, N], BF16, tag="q", name="q_d")
            proj_q(q_d, wt_d["wq"], wt_d["bq"], xs_bf)
            proj_q(q_e, wt_e["wq"], wt_e["bq"], tot_bf)

            def enc_post(gi):
                n0, gw = GROUPS[gi]
                proj_k(kp_d, wt_d["wk"], wt_d["pos"], xenc_bf, [(n0, gw)])
                proj_v(
                    vt_d, wt_d["wv"], xenc_bf, gamma_d,
                    range(n0 // P, (n0 + gw) // P),
                )

            attn(kp_e, q_e, vt_e, wt_e, "enc", post_group=enc_post)
            attn(kp_d, q_d, vt_d, wt_d, "dec")

    nc.compile()
    return nc


def kernel(**inputs):
    import ml_dtypes

    def to_bf(a):
        return np.ascontiguousarray(a).astype(ml_dtypes.bfloat16)

    x = np.asarray(inputs["x"], np.float32)
    total = np.asarray(inputs["total"], np.float32)

    def pack_cn(a_cn):
        out = np.empty((P, KC * a_cn.shape[1]), a_cn.dtype)
        M = a_cn.shape[1]
        for k in range(KC):
            out[:, k * M : (k + 1) * M] = a_cn[k * P : (k + 1) * P]
        return out

    def prep(pfx):
        Wq = np.asarray(inputs[f"{pfx}_Wq"], np.float32)
        bq = np.asarray(inputs[f"{pfx}_bq"], np.float32)
        Wk = np.asarray(inputs[f"{pfx}_Wk"], np.float32)
        bk = np.asarray(inputs[f"{pfx}_bk"], np.float32)
        Wv = np.asarray(inputs[f"{pfx}_Wv"], np.float32)
        bv = np.asarray(inputs[f"{pfx}_bv"], np.float32)
        ht = np.asarray(inputs[f"{pfx}_ht"], np.float32)
        wtt = np.asarray(inputs[f"{pfx}_wt"], np.float32)
        gamma = float(np.asarray(inputs[f"{pfx}_gamma"], np.float32).reshape(-1)[0])
        pos = (ht + wtt).reshape(C8, N) + bk[:, None]

        def dup_pack(wT):
            # [C, C8] -> [128, KC*128]: chunk k cols = [wT_k | wT_k]
            out = np.empty((P, KC * P), np.float32)
            for k in range(KC):
                blk = wT[k * P : (k + 1) * P]            # [128, 64]
                out[:, k * P : k * P + C8] = blk
                out[:, k * P + C8 : (k + 1) * P] = blk
            return out

        pos_dup = np.concatenate([pos, pos], axis=0)      # [128, N]
        bq_dup = np.concatenate([bq, bq]).reshape(P, 1)
        gvb = np.ascontiguousarray((gamma * bv).reshape(KC, P).T)  # [128, KC]
        return {
            "wk": to_bf(dup_pack(np.ascontiguousarray(Wk.T))),
            "wq": to_bf(dup_pack(np.ascontiguousarray(Wq.T))),
            "wv": to_bf(pack_cn(np.ascontiguousarray(Wv.T))),
            "pos": to_bf(pos_dup),
            "bq": np.ascontiguousarray(bq_dup),
            "gvb": gvb,
            "gamma": gamma,
        }

    pe, pd = prep("enc"), prep("dec")
    nc = build_bass(pe["gamma"], pd["gamma"])

    in_maps = []
    for b in range(B):
        x_cn = np.ascontiguousarray(x[b].reshape(C, N))
        tot_cn = np.ascontiguousarray(total[b].reshape(C, N))
        m = {
            "x_bf": to_bf(pack_cn(x_cn)),
            "tot_bf": to_bf(pack_cn(tot_cn)),
            "x_cn": x_cn,
        }
        for p, w in (("e", pe), ("d", pd)):
            for key in ("wk", "wq", "wv", "pos", "bq", "gvb"):
                m[f"{key}_{p}"] = w[key]
        in_maps.append(m)

    res = run_bass_kernel_spmd(nc, in_maps, core_ids=list(range(B)))
    out = np.stack(
        [res.results[b]["out_cn"].reshape(C, H, W) for b in range(B)], axis=0
    )
    return out.astype(np.float32)


if __name__ == "__main__":
    import reference

    ins = {k: np.asarray(v) for k, v in reference.setup_inputs().items()}
    got = kernel(**ins)
    exp = np.asarray(reference.reference(**ins))
    err = np.abs(got - exp).max() / (np.abs(exp).max() + 1e-30)
    print("abs-rel err:", err)
